# revision 57
# baseline (speedup 1.0000x reference)
"""Multi-head attention (B=2,T=2048,C=1024,H=16,RoPE,causal) on 8 TRN2 cores.

Sharding: core c -> (batch b = c//4, head-group g = c%4, heads [4g,4g+4)).
Each core computes QKV projection for its 4 heads against x[b], RoPE,
causal attention in transposed-score layout [s, t], and the output
projection rows t' in [512g, 512g+512) of y[b] (the reference's
(B,H,T,Dh)->(B,T,C) reshape makes output blocks head-disjoint).

Round-1 restructure vs. the original baseline (258.7us):
- stage A (proj+RoPE) and stage B (attention) interleaved per t-tile so
  the scheduler can fill stage-B exp bubbles with stage-A matmuls.
- input DMAs reordered (wqkv+x0 first, w_proj last) and spread across
  engine queues so compute starts ~7us in instead of ~20us.
- out-proj PSUM borrows the qk pool (no psA contention at hp boundary).
- attention output columns permuted to (j,k) order via a strided moving
  AP on the P@V matmul, making the out-proj stationary-gather reads
  contiguous; att results per (hp,hl) live in separate tiles.
- normalize: reciprocal straight off the PSUM ones-row.
- explicit engine routing (ACT=exp only; copies on DVE/Pool).
"""
import math
import sys

sys.path.insert(0, '/opt/trn_rl_repo')
sys.path.insert(0, '/opt/pypackages')

import ml_dtypes
import numpy as np
from contextlib import ExitStack

import concourse.bass as bass  # noqa: F401
import concourse.tile as tile
from concourse import bacc, mybir
from concourse.bass_utils import run_bass_kernel_spmd

BF16 = mybir.dt.bfloat16
F32 = mybir.dt.float32
NPBF16 = ml_dtypes.bfloat16

B, T, C, H, Dh = 2, 2048, 1024, 16, 64
HALF = Dh // 2          # 32
NCORES = 8
HPC = 4                 # heads per core
CPC = HPC * Dh          # channels per core = 256
SCALE = 1.0 / math.sqrt(Dh)
TT = 512                # t-tile width
NTT = T // TT           # 4
SC = 128                # s-chunk width

_compiled_nc = None


def _build_nc():
    nc = bacc.Bacc("TRN2", target_bir_lowering=False, debug=False)

    xT = nc.dram_tensor("xT", [C, T], BF16, kind="ExternalInput").ap()
    wqkvT = nc.dram_tensor("wqkvT", [C, 3 * CPC], BF16, kind="ExternalInput").ap()
    wpT = nc.dram_tensor("wpT", [C, C], BF16, kind="ExternalInput").ap()
    cosx = nc.dram_tensor("cosx", [128, T], BF16, kind="ExternalInput").ap()
    sinx = nc.dram_tensor("sinx", [128, T], BF16, kind="ExternalInput").ap()
    mask01 = nc.dram_tensor("mask01", [128, 128], BF16, kind="ExternalInput").ap()
    yblk = nc.dram_tensor("yblk", [512, C], BF16, kind="ExternalOutput").ap()

    with tile.TileContext(nc) as tc, ExitStack() as ctx:
        const = ctx.enter_context(tc.tile_pool(name="const", bufs=1))
        qkpool = ctx.enter_context(tc.tile_pool(name="qk", bufs=2))
        vpool = ctx.enter_context(tc.tile_pool(name="vnat", bufs=4))
        attp = ctx.enter_context(tc.tile_pool(name="attp", bufs=1))
        tmp = ctx.enter_context(tc.tile_pool(name="tmp", bufs=3))
        ahpool = ctx.enter_context(tc.tile_pool(name="ahp", bufs=17))
        psA = ctx.enter_context(tc.tile_pool(name="psA", bufs=2, space="PSUM"))
        psQK = ctx.enter_context(tc.tile_pool(name="psQK", bufs=2, space="PSUM"))
        psAT = ctx.enter_context(tc.tile_pool(name="psAT", bufs=2, space="PSUM"))

        # ---- constants; ordered/spread so compute starts early ----
        wqkv_sb = const.tile([128, 8, 3 * CPC], BF16)
        wqkvTr = wqkvT.rearrange("(cc p) f -> p cc f", p=128)
        # q rows land first so the first projection group isn't gated on
        # the whole 1.5MB; k+v follow on the same queue.
        nc.sync.dma_start(wqkv_sb[:, :, 0:CPC], wqkvTr[:, :, 0:CPC])
        nc.sync.dma_start(wqkv_sb[:, :, CPC:3 * CPC], wqkvTr[:, :, CPC:3 * CPC])
        x_sb = []
        for tt in range(NTT):
            xs = const.tile([128, 8, TT], BF16, name=f"x_sb{tt}")
            x_sb.append(xs)
        xTr = xT.rearrange("(cc p) t -> p cc t", p=128)
        # x tile 0 split across two queues so compute can start earliest
        nc.scalar.dma_start(x_sb[0][:, 0:4, :], xTr[:, 0:4, 0:TT])
        nc.gpsimd.dma_start(x_sb[0][:, 4:8, :], xTr[:, 4:8, 0:TT])
        cos_sb = const.tile([128, T], BF16)
        nc.scalar.dma_start(cos_sb[:], cosx[:])
        sin_sb = const.tile([128, T], BF16)
        nc.scalar.dma_start(sin_sb[:], sinx[:])
        mask_sb = const.tile([128, 128], BF16)
        nc.gpsimd.dma_start(mask_sb[:], mask01[:])
        nc.sync.dma_start(x_sb[1][:], xTr[:, :, TT:2 * TT])
        # x2/x3/w_proj DMAs are issued later, staged into the emission
        # stream, so they don't compete for HBM bandwidth with the loads
        # that gate the first projection tiles.
        wt_sb = const.tile([128, 8, C], BF16)

        # att results per (hp, hl): [64 d, T] bf16, natural t order
        att_sb = [[attp.tile([64, T], BF16, tag=f"att{hp}{hl}",
                             name=f"att_{hp}_{hl}")
                   for hl in range(2)] for hp in range(2)]

        # V for all 4 heads, natural [s, d] layout: [128 s-in-chunk,
        # 16 chunk, 4 head, 65] with a ones column for the denominator.
        v_nat = vpool.tile([128, T // SC, HPC, Dh + 1], BF16, name="v_nat")
        nc.gpsimd.memset(v_nat[:, :, :, Dh:Dh + 1], 1.0)

        qk_tiles = {}

        def get_qk(hp):
            if hp not in qk_tiles:
                qk_tiles[hp] = (qkpool.tile([128, T], BF16, tag="q",
                                            name=f"q_sb{hp}"),
                                qkpool.tile([128, T], BF16, tag="k",
                                            name=f"k_sb{hp}"))
            return qk_tiles[hp]

        if True:
            def emit_A(hp, tt):
                q_sb, k_sb = get_qk(hp)
                ts = slice(tt * TT, (tt + 1) * TT)
                # q/k projection + RoPE for this hp's two heads
                for gi, grp in enumerate(("q", "k")):
                    f0 = gi * CPC + hp * 128
                    gps = psA.tile([128, TT], F32, tag="mm",
                                   name=f"gps_{hp}_{tt}_{gi}")
                    for cc in range(8):
                        nc.tensor.matmul(
                            gps[:], wqkv_sb[:, cc, f0:f0 + 128],
                            x_sb[tt][:, cc, :],
                            start=(cc == 0), stop=(cc == 7))
                    gb = tmp.tile([128, TT], BF16, tag="gb")
                    nc.vector.tensor_copy(gb[:], gps[:])
                    gc = tmp.tile([128, TT], BF16, tag="gc")
                    nc.vector.tensor_mul(gc[:], gb[:], cos_sb[:, ts])
                    # rotate-half via partition-shifted muls; sinS has the
                    # rotation sign baked in per 32-row block.
                    gs = tmp.tile([128, TT], BF16, tag="gs")
                    # both SBUF inputs must share a base partition; only the
                    # output is partition-shifted.  sin_sb row block rin
                    # carries the sign required by OUTPUT block r0.
                    for r0, rin in ((0, 32), (32, 0), (64, 96), (96, 64)):
                        nc.vector.tensor_mul(
                            gs[r0:r0 + 32, :], gb[rin:rin + 32, :],
                            sin_sb[rin:rin + 32, ts])
                    dest = q_sb if grp == "q" else k_sb
                    nc.vector.tensor_add(dest[:, ts], gc[:], gs[:])
                if hp == 0:
                    # V^T projection for ALL 4 heads: x chunk stationary,
                    # wv moving -> psum [128 t, 256 (head,d)], no transpose.
                    for tc in range(TT // 128):
                        ci = tt * 4 + tc
                        vps = psA.tile([128, TT], F32, tag="mm",
                                       name=f"vps_{tt}_{tc}")
                        for cc in range(8):
                            nc.tensor.matmul(
                                vps[:, 0:256],
                                x_sb[tt][:, cc, tc * 128:(tc + 1) * 128],
                                wqkv_sb[:, cc, 2 * CPC:3 * CPC],
                                start=(cc == 0), stop=(cc == 7))
                        nc.any.tensor_copy(
                            v_nat[:, ci, :, 0:Dh],
                            vps[:, 0:256].rearrange("s (hd d) -> s hd d",
                                                    d=Dh))

            def emit_B(hp, tt):
                q_sb, k_sb = get_qk(hp)
                ts = slice(tt * TT, (tt + 1) * TT)
                outs = [psAT.tile([Dh + 1, TT], F32, tag="attps",
                                  name=f"attps_{hp}_{tt}_{hl}")
                        for hl in range(2)]
                njs = 4 * tt + 4

                def attv(j):
                    pb, off = pbs[j]
                    o = max(off, 0)
                    for hl in range(2):
                        nc.tensor.matmul(
                            outs[hl][:, o:TT], v_nat[:, j, hp * 2 + hl, :],
                            pb[:, hl * TT + o:(hl + 1) * TT],
                            start=(j == 0), stop=(j == njs - 1))

                pbs = {}
                for j in range(njs):
                    sj = slice(j * SC, (j + 1) * SC)
                    # diagonal chunks only need score columns t >= off
                    off = (j - 4 * tt) * 128 if j >= 4 * tt else -1
                    o = max(off, 0)
                    qk = psQK.tile([128, 2 * TT], F32, tag="qk",
                                   name=f"qk_{hp}_{tt}_{j}")
                    for hl in range(2):
                        hb = hl * 64
                        nc.tensor.matmul(
                            qk[:, hl * TT + o:(hl + 1) * TT],
                            k_sb[hb:hb + 64, sj],
                            q_sb[hb:hb + 64, tt * TT + o:(tt + 1) * TT],
                            start=True, stop=True)
                    pb = tmp.tile([128, 2 * TT], BF16, tag="probs")
                    pbs[j] = (pb, off)
                    if off <= 0:
                        # one paired exp over both heads
                        nc.scalar.activation(
                            pb[:], qk[:], mybir.ActivationFunctionType.Exp,
                            scale=SCALE)
                    else:
                        for hl in range(2):
                            h0 = hl * TT
                            nc.scalar.activation(
                                pb[:, h0 + off:h0 + TT], qk[:, h0 + off:h0 + TT],
                                mybir.ActivationFunctionType.Exp, scale=SCALE)
                    if off >= 0:
                        for hl in range(2):
                            dsl = slice(hl * TT + off, hl * TT + off + 128)
                            nc.vector.tensor_mul(pb[:, dsl], pb[:, dsl],
                                                 mask_sb[:])
                    # software pipeline, 2 deep: P@V for chunk j-2 lands
                    # after this chunk's score matmuls in PE program order.
                    # The 1-deep window (~850ns of PE work) is shorter than
                    # one exp (~1us), so 1-deep still stalled the PE a
                    # little on every chunk.
                    if j > 1:
                        attv(j - 2)
                if njs > 1:
                    attv(njs - 2)
                attv(njs - 1)
                # normalize and store to att_sb
                for hl in range(2):
                    op = outs[hl]
                    zrow = tmp.tile([1, TT], F32, tag="zrow")
                    nc.vector.tensor_copy(zrow[:], op[Dh:Dh + 1, :])
                    zi = tmp.tile([1, TT], F32, tag="zi")
                    nc.vector.reciprocal_approx_fast(out=zi[:], in_=zrow[:])
                    zb = tmp.tile([64, TT], F32, tag="zb")
                    nc.gpsimd.partition_broadcast(zb[:], zi[:], channels=64)
                    nc.vector.tensor_mul(
                        att_sb[hp][hl][:, ts], op[0:Dh, :], zb[:])

            def emit_P(hp):
                # ---- output projection for this hp's two heads ----
                # Reference reshapes (B,H,T,Dh) row-major into (B,T,C):
                # row t' = h*128 + k draws from head h positions t = 16k+j,
                # channel c' = 64j + d.  Y_h[k,o] = sum_c' A_hT[c',k] WT[c',o],
                # A_hT[64j+d, k] = attT_h[d, 16k+j].
                # Both heads' gathers are emitted before the matmuls so the
                # hl=1 gather latency hides under hl=0's projection matmuls.
                ahts = {}
                for hl in range(2):
                    att_v = att_sb[hp][hl][:].rearrange("d (k j) -> d k j",
                                                        j=16)
                    for cc in range(8):
                        aht = ahpool.tile([128, 128], BF16, tag="aht",
                                          name=f"aht_{hp}_{hl}_{cc}")
                        eng = nc.gpsimd if cc % 2 == 0 else nc.vector
                        eng.tensor_copy(aht[0:64, :], att_v[:, :, 2 * cc])
                        eng.tensor_copy(aht[64:128, :], att_v[:, :, 2 * cc + 1])
                        ahts[hl, cc] = aht
                for hl in range(2):
                    r0 = (hp * 2 + hl) * 128
                    ypss = psQK.tile([128, 2 * TT], F32, tag="qk",
                                     name=f"yps_{hp}_{hl}")
                    for cc in range(8):
                        for ot in range(2):
                            nc.tensor.matmul(
                                ypss[:, ot * 512:(ot + 1) * 512],
                                ahts[hl, cc][:],
                                wt_sb[:, cc, ot * 512:(ot + 1) * 512],
                                start=(cc == 0), stop=(cc == 7))
                    for ot in range(2):
                        yo = tmp.tile([128, 512], BF16, tag="yo")
                        nc.any.tensor_copy(yo[:],
                                           ypss[:, ot * 512:(ot + 1) * 512])
                        nc.sync.dma_start(
                            yblk[r0:r0 + 128, ot * 512:(ot + 1) * 512], yo[:])

            # Emission order: A one tile ahead of B within each hp (the
            # schedule is static list-order per engine, so critical-path
            # work must come first; hoisting independent work earlier
            # measurably delays it).  Late input DMAs are staged into the
            # stream; one hp1 A-tile slots before hp0's out-projection so
            # the PE has matmul work during the gather chain.
            emit_A(0, 0)
            emit_A(0, 1)
            emit_B(0, 0)
            nc.scalar.dma_start(x_sb[2][:], xTr[:, :, 2 * TT:3 * TT])
            emit_A(0, 2)
            emit_B(0, 1)
            nc.scalar.dma_start(x_sb[3][:], xTr[:, :, 3 * TT:4 * TT])
            emit_A(0, 3)
            emit_B(0, 2)
            nc.sync.dma_start(wt_sb[:],
                              wpT.rearrange("(cc p) o -> p cc o", p=128))
            emit_B(0, 3)
            emit_A(1, 0)
            emit_P(0)
            emit_A(1, 1)
            emit_B(1, 0)
            emit_A(1, 2)
            emit_B(1, 1)
            emit_A(1, 3)
            emit_B(1, 2)
            emit_B(1, 3)
            emit_P(1)

    nc.compile()
    return nc


def _get_nc():
    global _compiled_nc
    if _compiled_nc is None:
        _compiled_nc = _build_nc()
    return _compiled_nc


def _host_tables():
    pos = np.arange(T, dtype=np.float32)[:, None]
    inv = np.exp(np.arange(0, Dh, 2, dtype=np.float32)
                 * (-math.log(10000.0) / Dh))
    ang = pos * inv                       # (T, 32)
    sin, cos = np.sin(ang), np.cos(ang)   # (T, 32)
    idx = np.arange(128) % HALF           # d % 32
    cos_ext = cos[:, idx].T.astype(NPBF16)  # (128, T)
    # Rotation sign baked in, indexed by the INPUT row block of the
    # partition-shifted mul: out rows (0:32, 64:96) need -sin and read
    # input rows (32:64, 96:128); out rows (32:64, 96:128) need +sin and
    # read input rows (0:32, 64:96).
    sign = np.where((np.arange(128) % 64) < HALF, 1.0, -1.0)[:, None]
    sin_ext = (sign * sin[:, idx].T).astype(NPBF16)

    s_i = np.arange(128)[:, None]
    t_i = np.arange(128)[None, :]
    mask01 = (t_i >= s_i).astype(np.float32).astype(NPBF16)
    return cos_ext, sin_ext, mask01


def kernel(x, w_qkv, w_proj):
    x = np.asarray(x)
    w_qkv = np.asarray(w_qkv)
    w_proj = np.asarray(w_proj)
    nc = _get_nc()
    in_maps = build_in_maps(x, w_qkv, w_proj)
    res = run_bass_kernel_spmd(nc, in_maps, core_ids=list(range(NCORES)))
    y = np.zeros((B, T, C), dtype=np.float32)
    for c in range(NCORES):
        b, g = c // 4, c % 4
        y[b, 512 * g:512 * g + 512, :] = res.results[c]["yblk"].astype(
            np.float32)
    return y


def build_in_maps(x, w_qkv, w_proj):
    cos_ext, sin_ext, mask01 = _host_tables()
    wq4 = w_qkv.reshape(3, H, Dh, C)
    wpT = np.ascontiguousarray(w_proj.T.astype(NPBF16))
    in_maps = []
    for c in range(NCORES):
        b, g = c // 4, c % 4
        hs = slice(4 * g, 4 * g + 4)
        wq = wq4[0, hs].reshape(CPC, C)
        wk = wq4[1, hs].reshape(CPC, C)
        wv = wq4[2, hs].reshape(CPC, C)
        wqkvT = np.concatenate([wq, wk, wv], axis=0).T.astype(NPBF16)
        xT = x[b].T.astype(NPBF16)
        in_maps.append({
            "xT": np.ascontiguousarray(xT),
            "wqkvT": np.ascontiguousarray(wqkvT),
            "wpT": wpT,
            "cosx": cos_ext, "sinx": sin_ext,
            "mask01": mask01,
        })
    return in_maps


# revision 58
# speedup vs baseline: 1.0033x; 1.0033x over previous
"""Multi-head attention (B=2,T=2048,C=1024,H=16,RoPE,causal) on 8 TRN2 cores.

Sharding: core c -> (batch b = c//4, head-group g = c%4, heads [4g,4g+4)).
Each core computes QKV projection for its 4 heads against x[b], RoPE,
causal attention in transposed-score layout [s, t], and the output
projection rows t' in [512g, 512g+512) of y[b] (the reference's
(B,H,T,Dh)->(B,T,C) reshape makes output blocks head-disjoint).

Round-1 restructure vs. the original baseline (258.7us):
- stage A (proj+RoPE) and stage B (attention) interleaved per t-tile so
  the scheduler can fill stage-B exp bubbles with stage-A matmuls.
- input DMAs reordered (wqkv+x0 first, w_proj last) and spread across
  engine queues so compute starts ~7us in instead of ~20us.
- out-proj PSUM borrows the qk pool (no psA contention at hp boundary).
- attention output columns permuted to (j,k) order via a strided moving
  AP on the P@V matmul, making the out-proj stationary-gather reads
  contiguous; att results per (hp,hl) live in separate tiles.
- normalize: reciprocal straight off the PSUM ones-row.
- explicit engine routing (ACT=exp only; copies on DVE/Pool).
"""
import math
import sys

sys.path.insert(0, '/opt/trn_rl_repo')
sys.path.insert(0, '/opt/pypackages')

import ml_dtypes
import numpy as np
from contextlib import ExitStack

import concourse.bass as bass  # noqa: F401
import concourse.tile as tile
from concourse import bacc, mybir
from concourse.bass_utils import run_bass_kernel_spmd

BF16 = mybir.dt.bfloat16
F32 = mybir.dt.float32
NPBF16 = ml_dtypes.bfloat16

B, T, C, H, Dh = 2, 2048, 1024, 16, 64
HALF = Dh // 2          # 32
NCORES = 8
HPC = 4                 # heads per core
CPC = HPC * Dh          # channels per core = 256
SCALE = 1.0 / math.sqrt(Dh)
TT = 512                # t-tile width
NTT = T // TT           # 4
SC = 128                # s-chunk width

_compiled_nc = None


def _build_nc():
    nc = bacc.Bacc("TRN2", target_bir_lowering=False, debug=False)

    xT = nc.dram_tensor("xT", [C, T], BF16, kind="ExternalInput").ap()
    wqkvT = nc.dram_tensor("wqkvT", [C, 3 * CPC], BF16, kind="ExternalInput").ap()
    wpT = nc.dram_tensor("wpT", [C, C], BF16, kind="ExternalInput").ap()
    cosx = nc.dram_tensor("cosx", [128, T], BF16, kind="ExternalInput").ap()
    sinx = nc.dram_tensor("sinx", [128, T], BF16, kind="ExternalInput").ap()
    mask01 = nc.dram_tensor("mask01", [128, 128], BF16, kind="ExternalInput").ap()
    yblk = nc.dram_tensor("yblk", [512, C], BF16, kind="ExternalOutput").ap()

    with tile.TileContext(nc) as tc, ExitStack() as ctx:
        const = ctx.enter_context(tc.tile_pool(name="const", bufs=1))
        qkpool = ctx.enter_context(tc.tile_pool(name="qk", bufs=2))
        vpool = ctx.enter_context(tc.tile_pool(name="vnat", bufs=4))
        attp = ctx.enter_context(tc.tile_pool(name="attp", bufs=1))
        tmp = ctx.enter_context(tc.tile_pool(name="tmp", bufs=3))
        ahpool = ctx.enter_context(tc.tile_pool(name="ahp", bufs=17))
        psA = ctx.enter_context(tc.tile_pool(name="psA", bufs=2, space="PSUM"))
        psQK = ctx.enter_context(tc.tile_pool(name="psQK", bufs=2, space="PSUM"))
        psAT = ctx.enter_context(tc.tile_pool(name="psAT", bufs=2, space="PSUM"))

        # ---- constants; ordered/spread so compute starts early ----
        wqkv_sb = const.tile([128, 8, 3 * CPC], BF16)
        wqkvTr = wqkvT.rearrange("(cc p) f -> p cc f", p=128)
        # q rows land first so the first projection group isn't gated on
        # the whole 1.5MB; k+v follow on the same queue.
        nc.sync.dma_start(wqkv_sb[:, :, 0:CPC], wqkvTr[:, :, 0:CPC])
        nc.sync.dma_start(wqkv_sb[:, :, CPC:3 * CPC], wqkvTr[:, :, CPC:3 * CPC])
        x_sb = []
        for tt in range(NTT):
            xs = const.tile([128, 8, TT], BF16, name=f"x_sb{tt}")
            x_sb.append(xs)
        xTr = xT.rearrange("(cc p) t -> p cc t", p=128)
        # x tile 0 split across two queues so compute can start earliest
        nc.scalar.dma_start(x_sb[0][:, 0:4, :], xTr[:, 0:4, 0:TT])
        nc.gpsimd.dma_start(x_sb[0][:, 4:8, :], xTr[:, 4:8, 0:TT])
        cos_sb = const.tile([128, T], BF16)
        nc.scalar.dma_start(cos_sb[:], cosx[:])
        sin_sb = const.tile([128, T], BF16)
        nc.scalar.dma_start(sin_sb[:], sinx[:])
        mask_sb = const.tile([128, 128], BF16)
        nc.gpsimd.dma_start(mask_sb[:], mask01[:])
        nc.sync.dma_start(x_sb[1][:], xTr[:, :, TT:2 * TT])
        # x2/x3/w_proj DMAs are issued later, staged into the emission
        # stream, so they don't compete for HBM bandwidth with the loads
        # that gate the first projection tiles.
        wt_sb = const.tile([128, 8, C], BF16)

        # att results per (hp, hl): [64 d, T] bf16, natural t order
        att_sb = [[attp.tile([64, T], BF16, tag=f"att{hp}{hl}",
                             name=f"att_{hp}_{hl}")
                   for hl in range(2)] for hp in range(2)]

        # V for all 4 heads, natural [s, d] layout: [128 s-in-chunk,
        # 16 chunk, 4 head, 65] with a ones column for the denominator.
        v_nat = vpool.tile([128, T // SC, HPC, Dh + 1], BF16, name="v_nat")
        nc.gpsimd.memset(v_nat[:, :, :, Dh:Dh + 1], 1.0)

        qk_tiles = {}

        def get_qk(hp):
            if hp not in qk_tiles:
                qk_tiles[hp] = (qkpool.tile([128, T], BF16, tag="q",
                                            name=f"q_sb{hp}"),
                                qkpool.tile([128, T], BF16, tag="k",
                                            name=f"k_sb{hp}"))
            return qk_tiles[hp]

        if True:
            def emit_A(hp, tt):
                q_sb, k_sb = get_qk(hp)
                ts = slice(tt * TT, (tt + 1) * TT)
                # q/k projection + RoPE for this hp's two heads
                for gi, grp in enumerate(("q", "k")):
                    f0 = gi * CPC + hp * 128
                    gps = psA.tile([128, TT], F32, tag="mm",
                                   name=f"gps_{hp}_{tt}_{gi}")
                    for cc in range(8):
                        nc.tensor.matmul(
                            gps[:], wqkv_sb[:, cc, f0:f0 + 128],
                            x_sb[tt][:, cc, :],
                            start=(cc == 0), stop=(cc == 7))
                    gb = tmp.tile([128, TT], BF16, tag="gb")
                    nc.vector.tensor_copy(gb[:], gps[:])
                    gc = tmp.tile([128, TT], BF16, tag="gc")
                    nc.vector.tensor_mul(gc[:], gb[:], cos_sb[:, ts])
                    # rotate-half via partition-shifted muls; sinS has the
                    # rotation sign baked in per 32-row block.
                    gs = tmp.tile([128, TT], BF16, tag="gs")
                    # both SBUF inputs must share a base partition; only the
                    # output is partition-shifted.  sin_sb row block rin
                    # carries the sign required by OUTPUT block r0.
                    for r0, rin in ((0, 32), (32, 0), (64, 96), (96, 64)):
                        nc.vector.tensor_mul(
                            gs[r0:r0 + 32, :], gb[rin:rin + 32, :],
                            sin_sb[rin:rin + 32, ts])
                    dest = q_sb if grp == "q" else k_sb
                    nc.vector.tensor_add(dest[:, ts], gc[:], gs[:])
                if hp == 0:
                    # V^T projection for ALL 4 heads: x chunk stationary,
                    # wv moving -> psum [128 t, 256 (head,d)], no transpose.
                    for tc in range(TT // 128):
                        ci = tt * 4 + tc
                        vps = psA.tile([128, TT], F32, tag="mm",
                                       name=f"vps_{tt}_{tc}")
                        for cc in range(8):
                            nc.tensor.matmul(
                                vps[:, 0:256],
                                x_sb[tt][:, cc, tc * 128:(tc + 1) * 128],
                                wqkv_sb[:, cc, 2 * CPC:3 * CPC],
                                start=(cc == 0), stop=(cc == 7))
                        nc.any.tensor_copy(
                            v_nat[:, ci, :, 0:Dh],
                            vps[:, 0:256].rearrange("s (hd d) -> s hd d",
                                                    d=Dh))

            def emit_B(hp, tt):
                q_sb, k_sb = get_qk(hp)
                ts = slice(tt * TT, (tt + 1) * TT)
                outs = [psAT.tile([Dh + 1, TT], F32, tag="attps",
                                  name=f"attps_{hp}_{tt}_{hl}")
                        for hl in range(2)]
                njs = 4 * tt + 4

                def attv(j):
                    pb, off = pbs[j]
                    o = max(off, 0)
                    for hl in range(2):
                        nc.tensor.matmul(
                            outs[hl][:, o:TT], v_nat[:, j, hp * 2 + hl, :],
                            pb[:, hl * TT + o:(hl + 1) * TT],
                            start=(j == 0), stop=(j == njs - 1))

                pbs = {}
                for j in range(njs):
                    sj = slice(j * SC, (j + 1) * SC)
                    # diagonal chunks only need score columns t >= off
                    off = (j - 4 * tt) * 128 if j >= 4 * tt else -1
                    o = max(off, 0)
                    qk = psQK.tile([128, 2 * TT], F32, tag="qk",
                                   name=f"qk_{hp}_{tt}_{j}")
                    for hl in range(2):
                        hb = hl * 64
                        nc.tensor.matmul(
                            qk[:, hl * TT + o:(hl + 1) * TT],
                            k_sb[hb:hb + 64, sj],
                            q_sb[hb:hb + 64, tt * TT + o:(tt + 1) * TT],
                            start=True, stop=True)
                    pb = tmp.tile([128, 2 * TT], BF16, tag="probs")
                    pbs[j] = (pb, off)
                    if off <= 0:
                        # one paired exp over both heads
                        nc.scalar.activation(
                            pb[:], qk[:], mybir.ActivationFunctionType.Exp,
                            scale=SCALE)
                    else:
                        for hl in range(2):
                            h0 = hl * TT
                            nc.scalar.activation(
                                pb[:, h0 + off:h0 + TT], qk[:, h0 + off:h0 + TT],
                                mybir.ActivationFunctionType.Exp, scale=SCALE)
                    if off >= 0:
                        for hl in range(2):
                            dsl = slice(hl * TT + off, hl * TT + off + 128)
                            nc.vector.tensor_mul(pb[:, dsl], pb[:, dsl],
                                                 mask_sb[:])
                    # software pipeline, 2 deep: P@V for chunk j-2 lands
                    # after this chunk's score matmuls in PE program order.
                    # The 1-deep window (~850ns of PE work) is shorter than
                    # one exp (~1us), so 1-deep still stalled the PE a
                    # little on every chunk.
                    if j > 1:
                        attv(j - 2)
                if njs > 1:
                    attv(njs - 2)
                attv(njs - 1)
                # normalize and store to att_sb
                for hl in range(2):
                    op = outs[hl]
                    zrow = tmp.tile([1, TT], F32, tag="zrow")
                    nc.vector.tensor_copy(zrow[:], op[Dh:Dh + 1, :])
                    zi = tmp.tile([1, TT], F32, tag="zi")
                    nc.vector.reciprocal_approx_fast(out=zi[:], in_=zrow[:])
                    zb = tmp.tile([64, TT], F32, tag="zb")
                    nc.gpsimd.partition_broadcast(zb[:], zi[:], channels=64)
                    nc.vector.tensor_mul(
                        att_sb[hp][hl][:, ts], op[0:Dh, :], zb[:])

            def emit_P(hp):
                # ---- output projection for this hp's two heads ----
                # Reference reshapes (B,H,T,Dh) row-major into (B,T,C):
                # row t' = h*128 + k draws from head h positions t = 16k+j,
                # channel c' = 64j + d.  Y_h[k,o] = sum_c' A_hT[c',k] WT[c',o],
                # A_hT[64j+d, k] = attT_h[d, 16k+j].
                # Both heads' gathers are emitted before the matmuls so the
                # hl=1 gather latency hides under hl=0's projection matmuls.
                ahts = {}
                for hl in range(2):
                    att_v = att_sb[hp][hl][:].rearrange("d (k j) -> d k j",
                                                        j=16)
                    for cc in range(8):
                        aht = ahpool.tile([128, 128], BF16, tag="aht",
                                          name=f"aht_{hp}_{hl}_{cc}")
                        eng = nc.gpsimd if cc % 2 == 0 else nc.vector
                        eng.tensor_copy(aht[0:64, :], att_v[:, :, 2 * cc])
                        eng.tensor_copy(aht[64:128, :], att_v[:, :, 2 * cc + 1])
                        ahts[hl, cc] = aht
                for hl in range(2):
                    r0 = (hp * 2 + hl) * 128
                    ypss = psQK.tile([128, 2 * TT], F32, tag="qk",
                                     name=f"yps_{hp}_{hl}")
                    for cc in range(8):
                        for ot in range(2):
                            nc.tensor.matmul(
                                ypss[:, ot * 512:(ot + 1) * 512],
                                ahts[hl, cc][:],
                                wt_sb[:, cc, ot * 512:(ot + 1) * 512],
                                start=(cc == 0), stop=(cc == 7))
                    for ot in range(2):
                        yo = tmp.tile([128, 512], BF16, tag="yo")
                        nc.any.tensor_copy(yo[:],
                                           ypss[:, ot * 512:(ot + 1) * 512])
                        nc.sync.dma_start(
                            yblk[r0:r0 + 128, ot * 512:(ot + 1) * 512], yo[:])

            # Emission order: A one tile ahead of B within each hp (the
            # schedule is static list-order per engine, so critical-path
            # work must come first; hoisting independent work earlier
            # measurably delays it).  Late input DMAs are staged into the
            # stream; one hp1 A-tile slots before hp0's out-projection so
            # the PE has matmul work during the gather chain.
            emit_A(0, 0)
            emit_A(0, 1)
            emit_B(0, 0)
            nc.scalar.dma_start(x_sb[2][:], xTr[:, :, 2 * TT:3 * TT])
            emit_A(0, 2)
            emit_B(0, 1)
            nc.scalar.dma_start(x_sb[3][:], xTr[:, :, 3 * TT:4 * TT])
            emit_A(0, 3)
            emit_B(0, 2)
            nc.sync.dma_start(wt_sb[:],
                              wpT.rearrange("(cc p) o -> p cc o", p=128))
            emit_B(0, 3)
            emit_P(0)
            emit_A(1, 0)
            emit_A(1, 1)
            emit_B(1, 0)
            emit_A(1, 2)
            emit_B(1, 1)
            emit_A(1, 3)
            emit_B(1, 2)
            emit_B(1, 3)
            emit_P(1)

    nc.compile()
    return nc


def _get_nc():
    global _compiled_nc
    if _compiled_nc is None:
        _compiled_nc = _build_nc()
    return _compiled_nc


def _host_tables():
    pos = np.arange(T, dtype=np.float32)[:, None]
    inv = np.exp(np.arange(0, Dh, 2, dtype=np.float32)
                 * (-math.log(10000.0) / Dh))
    ang = pos * inv                       # (T, 32)
    sin, cos = np.sin(ang), np.cos(ang)   # (T, 32)
    idx = np.arange(128) % HALF           # d % 32
    cos_ext = cos[:, idx].T.astype(NPBF16)  # (128, T)
    # Rotation sign baked in, indexed by the INPUT row block of the
    # partition-shifted mul: out rows (0:32, 64:96) need -sin and read
    # input rows (32:64, 96:128); out rows (32:64, 96:128) need +sin and
    # read input rows (0:32, 64:96).
    sign = np.where((np.arange(128) % 64) < HALF, 1.0, -1.0)[:, None]
    sin_ext = (sign * sin[:, idx].T).astype(NPBF16)

    s_i = np.arange(128)[:, None]
    t_i = np.arange(128)[None, :]
    mask01 = (t_i >= s_i).astype(np.float32).astype(NPBF16)
    return cos_ext, sin_ext, mask01


def kernel(x, w_qkv, w_proj):
    x = np.asarray(x)
    w_qkv = np.asarray(w_qkv)
    w_proj = np.asarray(w_proj)
    nc = _get_nc()
    in_maps = build_in_maps(x, w_qkv, w_proj)
    res = run_bass_kernel_spmd(nc, in_maps, core_ids=list(range(NCORES)))
    y = np.zeros((B, T, C), dtype=np.float32)
    for c in range(NCORES):
        b, g = c // 4, c % 4
        y[b, 512 * g:512 * g + 512, :] = res.results[c]["yblk"].astype(
            np.float32)
    return y


def build_in_maps(x, w_qkv, w_proj):
    cos_ext, sin_ext, mask01 = _host_tables()
    wq4 = w_qkv.reshape(3, H, Dh, C)
    wpT = np.ascontiguousarray(w_proj.T.astype(NPBF16))
    in_maps = []
    for c in range(NCORES):
        b, g = c // 4, c % 4
        hs = slice(4 * g, 4 * g + 4)
        wq = wq4[0, hs].reshape(CPC, C)
        wk = wq4[1, hs].reshape(CPC, C)
        wv = wq4[2, hs].reshape(CPC, C)
        wqkvT = np.concatenate([wq, wk, wv], axis=0).T.astype(NPBF16)
        xT = x[b].T.astype(NPBF16)
        in_maps.append({
            "xT": np.ascontiguousarray(xT),
            "wqkvT": np.ascontiguousarray(wqkvT),
            "wpT": wpT,
            "cosx": cos_ext, "sinx": sin_ext,
            "mask01": mask01,
        })
    return in_maps


# revision 60
# speedup vs baseline: 1.0224x; 1.0190x over previous
"""Multi-head attention (B=2,T=2048,C=1024,H=16,RoPE,causal) on 8 TRN2 cores.

Sharding: core c -> (batch b = c//4, head-group g = c%4, heads [4g,4g+4)).
Each core computes QKV projection for its 4 heads against x[b], RoPE,
causal attention in transposed-score layout [s, t], and the output
projection rows t' in [512g, 512g+512) of y[b] (the reference's
(B,H,T,Dh)->(B,T,C) reshape makes output blocks head-disjoint).

Round-1 restructure vs. the original baseline (258.7us):
- stage A (proj+RoPE) and stage B (attention) interleaved per t-tile so
  the scheduler can fill stage-B exp bubbles with stage-A matmuls.
- input DMAs reordered (wqkv+x0 first, w_proj last) and spread across
  engine queues so compute starts ~7us in instead of ~20us.
- out-proj PSUM borrows the qk pool (no psA contention at hp boundary).
- attention output columns permuted to (j,k) order via a strided moving
  AP on the P@V matmul, making the out-proj stationary-gather reads
  contiguous; att results per (hp,hl) live in separate tiles.
- normalize: reciprocal straight off the PSUM ones-row.
- explicit engine routing (ACT=exp only; copies on DVE/Pool).
"""
import math
import sys

sys.path.insert(0, '/opt/trn_rl_repo')
sys.path.insert(0, '/opt/pypackages')

import ml_dtypes
import numpy as np
from contextlib import ExitStack

import concourse.bass as bass  # noqa: F401
import concourse.tile as tile
from concourse import bacc, mybir
from concourse.bass_utils import run_bass_kernel_spmd

BF16 = mybir.dt.bfloat16
F32 = mybir.dt.float32
NPBF16 = ml_dtypes.bfloat16

B, T, C, H, Dh = 2, 2048, 1024, 16, 64
HALF = Dh // 2          # 32
NCORES = 8
HPC = 4                 # heads per core
CPC = HPC * Dh          # channels per core = 256
SCALE = 1.0 / math.sqrt(Dh)
TT = 512                # t-tile width
NTT = T // TT           # 4
SC = 128                # s-chunk width

_compiled_nc = None


def _build_nc():
    nc = bacc.Bacc("TRN2", target_bir_lowering=False, debug=False)

    xT = nc.dram_tensor("xT", [C, T], BF16, kind="ExternalInput").ap()
    wqkvT = nc.dram_tensor("wqkvT", [C, 3 * CPC], BF16, kind="ExternalInput").ap()
    wpT = nc.dram_tensor("wpT", [C, C], BF16, kind="ExternalInput").ap()
    cosx = nc.dram_tensor("cosx", [128, T], BF16, kind="ExternalInput").ap()
    sinx = nc.dram_tensor("sinx", [128, T], BF16, kind="ExternalInput").ap()
    mask01 = nc.dram_tensor("mask01", [128, 128], BF16, kind="ExternalInput").ap()
    yblk = nc.dram_tensor("yblk", [512, C], BF16, kind="ExternalOutput").ap()

    with tile.TileContext(nc) as tc, ExitStack() as ctx:
        const = ctx.enter_context(tc.tile_pool(name="const", bufs=1))
        qkpool = ctx.enter_context(tc.tile_pool(name="qk", bufs=2))
        vpool = ctx.enter_context(tc.tile_pool(name="vnat", bufs=4))
        attp = ctx.enter_context(tc.tile_pool(name="attp", bufs=1))
        tmp = ctx.enter_context(tc.tile_pool(name="tmp", bufs=3))
        ahpool = ctx.enter_context(tc.tile_pool(name="ahp", bufs=17))
        psA = ctx.enter_context(tc.tile_pool(name="psA", bufs=2, space="PSUM"))
        psQK = ctx.enter_context(tc.tile_pool(name="psQK", bufs=2, space="PSUM"))
        psAT = ctx.enter_context(tc.tile_pool(name="psAT", bufs=2, space="PSUM"))

        # ---- constants; ordered/spread so compute starts early ----
        wqkv_sb = const.tile([128, 8, 3 * CPC], BF16)
        wqkvTr = wqkvT.rearrange("(cc p) f -> p cc f", p=128)
        # hp0's q rows land first so the very first projection group is
        # gated on only 0.25MB of weights; the rest follows on the queue.
        nc.sync.dma_start(wqkv_sb[:, :, 0:128], wqkvTr[:, :, 0:128])
        nc.sync.dma_start(wqkv_sb[:, :, 128:CPC], wqkvTr[:, :, 128:CPC])
        nc.sync.dma_start(wqkv_sb[:, :, CPC:3 * CPC], wqkvTr[:, :, CPC:3 * CPC])
        x_sb = []
        for tt in range(NTT):
            xs = const.tile([128, 8, TT], BF16, name=f"x_sb{tt}")
            x_sb.append(xs)
        xTr = xT.rearrange("(cc p) t -> p cc t", p=128)
        # x tile 0 split in quarters across two queues so the first
        # cc-chunks of the first matmul group can start earliest
        nc.scalar.dma_start(x_sb[0][:, 0:2, :], xTr[:, 0:2, 0:TT])
        nc.scalar.dma_start(x_sb[0][:, 2:4, :], xTr[:, 2:4, 0:TT])
        nc.gpsimd.dma_start(x_sb[0][:, 4:6, :], xTr[:, 4:6, 0:TT])
        nc.gpsimd.dma_start(x_sb[0][:, 6:8, :], xTr[:, 6:8, 0:TT])
        cos_sb = const.tile([128, T], BF16)
        nc.scalar.dma_start(cos_sb[:], cosx[:])
        sin_sb = const.tile([128, T], BF16)
        nc.scalar.dma_start(sin_sb[:], sinx[:])
        mask_sb = const.tile([128, 128], BF16)
        nc.gpsimd.dma_start(mask_sb[:], mask01[:])
        nc.sync.dma_start(x_sb[1][:], xTr[:, :, TT:2 * TT])
        # x2/x3/w_proj DMAs are issued later, staged into the emission
        # stream, so they don't compete for HBM bandwidth with the loads
        # that gate the first projection tiles.
        wt_sb = const.tile([128, 8, C], BF16)

        # att results per (hp, hl): [64 d, T] bf16, natural t order
        att_sb = [[attp.tile([64, T], BF16, tag=f"att{hp}{hl}",
                             name=f"att_{hp}_{hl}")
                   for hl in range(2)] for hp in range(2)]

        # V for all 4 heads, natural [s, d] layout: [128 s-in-chunk,
        # 16 chunk, 4 head, 65] with a ones column for the denominator.
        v_nat = vpool.tile([128, T // SC, HPC, Dh + 1], BF16, name="v_nat")
        nc.gpsimd.memset(v_nat[:, :, :, Dh:Dh + 1], 1.0)

        qk_tiles = {}

        def get_qk(hp):
            if hp not in qk_tiles:
                qk_tiles[hp] = (qkpool.tile([128, T], BF16, tag="q",
                                            name=f"q_sb{hp}"),
                                qkpool.tile([128, T], BF16, tag="k",
                                            name=f"k_sb{hp}"))
            return qk_tiles[hp]

        if True:
            def emit_A(hp, tt):
                q_sb, k_sb = get_qk(hp)
                ts = slice(tt * TT, (tt + 1) * TT)
                # q/k projection + RoPE for this hp's two heads
                for gi, grp in enumerate(("q", "k")):
                    f0 = gi * CPC + hp * 128
                    gps = psA.tile([128, TT], F32, tag="mm",
                                   name=f"gps_{hp}_{tt}_{gi}")
                    for cc in range(8):
                        nc.tensor.matmul(
                            gps[:], wqkv_sb[:, cc, f0:f0 + 128],
                            x_sb[tt][:, cc, :],
                            start=(cc == 0), stop=(cc == 7))
                    gb = tmp.tile([128, TT], BF16, tag="gb")
                    nc.vector.tensor_copy(gb[:], gps[:])
                    gc = tmp.tile([128, TT], BF16, tag="gc")
                    nc.vector.tensor_mul(gc[:], gb[:], cos_sb[:, ts])
                    # rotate-half via partition-shifted muls; sinS has the
                    # rotation sign baked in per 32-row block.
                    gs = tmp.tile([128, TT], BF16, tag="gs")
                    # both SBUF inputs must share a base partition; only the
                    # output is partition-shifted.  sin_sb row block rin
                    # carries the sign required by OUTPUT block r0.
                    for r0, rin in ((0, 32), (32, 0), (64, 96), (96, 64)):
                        nc.vector.tensor_mul(
                            gs[r0:r0 + 32, :], gb[rin:rin + 32, :],
                            sin_sb[rin:rin + 32, ts])
                    dest = q_sb if grp == "q" else k_sb
                    nc.vector.tensor_add(dest[:, ts], gc[:], gs[:])
                if hp == 0:
                    # V^T projection for ALL 4 heads: x chunk stationary,
                    # wv moving -> psum [128 t, 256 (head,d)], no transpose.
                    for tc in range(TT // 128):
                        ci = tt * 4 + tc
                        vps = psA.tile([128, TT], F32, tag="mm",
                                       name=f"vps_{tt}_{tc}")
                        for cc in range(8):
                            nc.tensor.matmul(
                                vps[:, 0:256],
                                x_sb[tt][:, cc, tc * 128:(tc + 1) * 128],
                                wqkv_sb[:, cc, 2 * CPC:3 * CPC],
                                start=(cc == 0), stop=(cc == 7))
                        nc.any.tensor_copy(
                            v_nat[:, ci, :, 0:Dh],
                            vps[:, 0:256].rearrange("s (hd d) -> s hd d",
                                                    d=Dh))

            def emit_B(hp, tt):
                q_sb, k_sb = get_qk(hp)
                ts = slice(tt * TT, (tt + 1) * TT)
                outs = [psAT.tile([Dh + 1, TT], F32, tag="attps",
                                  name=f"attps_{hp}_{tt}_{hl}")
                        for hl in range(2)]
                njs = 4 * tt + 4

                def attv(j):
                    pb, off = pbs[j]
                    o = max(off, 0)
                    for hl in range(2):
                        nc.tensor.matmul(
                            outs[hl][:, o:TT], v_nat[:, j, hp * 2 + hl, :],
                            pb[:, hl * TT + o:(hl + 1) * TT],
                            start=(j == 0), stop=(j == njs - 1))

                pbs = {}
                for j in range(njs):
                    sj = slice(j * SC, (j + 1) * SC)
                    # diagonal chunks only need score columns t >= off
                    off = (j - 4 * tt) * 128 if j >= 4 * tt else -1
                    o = max(off, 0)
                    qk = psQK.tile([128, 2 * TT], F32, tag="qk",
                                   name=f"qk_{hp}_{tt}_{j}")
                    for hl in range(2):
                        hb = hl * 64
                        nc.tensor.matmul(
                            qk[:, hl * TT + o:(hl + 1) * TT],
                            k_sb[hb:hb + 64, sj],
                            q_sb[hb:hb + 64, tt * TT + o:(tt + 1) * TT],
                            start=True, stop=True)
                    pb = tmp.tile([128, 2 * TT], BF16, tag="probs")
                    pbs[j] = (pb, off)
                    if off <= 0:
                        # one paired exp over both heads
                        nc.scalar.activation(
                            pb[:], qk[:], mybir.ActivationFunctionType.Exp,
                            scale=SCALE)
                    else:
                        for hl in range(2):
                            h0 = hl * TT
                            nc.scalar.activation(
                                pb[:, h0 + off:h0 + TT], qk[:, h0 + off:h0 + TT],
                                mybir.ActivationFunctionType.Exp, scale=SCALE)
                    if off >= 0:
                        for hl in range(2):
                            dsl = slice(hl * TT + off, hl * TT + off + 128)
                            nc.vector.tensor_mul(pb[:, dsl], pb[:, dsl],
                                                 mask_sb[:])
                    # software pipeline, 2 deep: P@V for chunk j-2 lands
                    # after this chunk's score matmuls in PE program order.
                    # The 1-deep window (~850ns of PE work) is shorter than
                    # one exp (~1us), so 1-deep still stalled the PE a
                    # little on every chunk.
                    if j > 1:
                        attv(j - 2)
                if njs > 1:
                    attv(njs - 2)
                attv(njs - 1)
                # normalize and store to att_sb
                for hl in range(2):
                    op = outs[hl]
                    zrow = tmp.tile([1, TT], F32, tag="zrow")
                    nc.vector.tensor_copy(zrow[:], op[Dh:Dh + 1, :])
                    zi = tmp.tile([1, TT], F32, tag="zi")
                    nc.vector.reciprocal_approx_fast(out=zi[:], in_=zrow[:])
                    zb = tmp.tile([64, TT], F32, tag="zb")
                    nc.gpsimd.partition_broadcast(zb[:], zi[:], channels=64)
                    nc.vector.tensor_mul(
                        att_sb[hp][hl][:, ts], op[0:Dh, :], zb[:])

            def emit_P(hp):
                # ---- output projection for this hp's two heads ----
                # Reference reshapes (B,H,T,Dh) row-major into (B,T,C):
                # row t' = h*128 + k draws from head h positions t = 16k+j,
                # channel c' = 64j + d.  Y_h[k,o] = sum_c' A_hT[c',k] WT[c',o],
                # A_hT[64j+d, k] = attT_h[d, 16k+j].
                # Both heads' gathers are emitted before the matmuls so the
                # hl=1 gather latency hides under hl=0's projection matmuls.
                ahts = {}
                for hl in range(2):
                    att_v = att_sb[hp][hl][:].rearrange("d (k j) -> d k j",
                                                        j=16)
                    for cc in range(8):
                        aht = ahpool.tile([128, 128], BF16, tag="aht",
                                          name=f"aht_{hp}_{hl}_{cc}")
                        if cc % 3 == 2:
                            # ACT is idle during the out-projection; use it
                            # as a third gather engine.
                            nc.scalar.activation(
                                aht[0:64, :], att_v[:, :, 2 * cc],
                                mybir.ActivationFunctionType.Copy)
                            nc.scalar.activation(
                                aht[64:128, :], att_v[:, :, 2 * cc + 1],
                                mybir.ActivationFunctionType.Copy)
                        else:
                            eng = nc.gpsimd if cc % 3 == 0 else nc.vector
                            eng.tensor_copy(aht[0:64, :], att_v[:, :, 2 * cc])
                            eng.tensor_copy(aht[64:128, :],
                                            att_v[:, :, 2 * cc + 1])
                        ahts[hl, cc] = aht
                for hl in range(2):
                    r0 = (hp * 2 + hl) * 128
                    ypss = psQK.tile([128, 2 * TT], F32, tag="qk",
                                     name=f"yps_{hp}_{hl}")
                    for cc in range(8):
                        for ot in range(2):
                            nc.tensor.matmul(
                                ypss[:, ot * 512:(ot + 1) * 512],
                                ahts[hl, cc][:],
                                wt_sb[:, cc, ot * 512:(ot + 1) * 512],
                                start=(cc == 0), stop=(cc == 7))
                    for ot in range(2):
                        yo = tmp.tile([128, 512], BF16, tag="yo")
                        nc.any.tensor_copy(yo[:],
                                           ypss[:, ot * 512:(ot + 1) * 512])
                        nc.sync.dma_start(
                            yblk[r0:r0 + 128, ot * 512:(ot + 1) * 512], yo[:])

            # Emission order: A one tile ahead of B within each hp (the
            # schedule is static list-order per engine, so critical-path
            # work must come first; hoisting independent work earlier
            # measurably delays it).  Late input DMAs are staged into the
            # stream; one hp1 A-tile slots before hp0's out-projection so
            # the PE has matmul work during the gather chain.
            emit_A(0, 0)
            emit_A(0, 1)
            emit_B(0, 0)
            nc.scalar.dma_start(x_sb[2][:], xTr[:, :, 2 * TT:3 * TT])
            emit_A(0, 2)
            emit_B(0, 1)
            nc.scalar.dma_start(x_sb[3][:], xTr[:, :, 3 * TT:4 * TT])
            emit_A(0, 3)
            emit_B(0, 2)
            nc.sync.dma_start(wt_sb[:],
                              wpT.rearrange("(cc p) o -> p cc o", p=128))
            emit_B(0, 3)
            emit_P(0)
            emit_A(1, 0)
            emit_A(1, 1)
            emit_B(1, 0)
            emit_A(1, 2)
            emit_B(1, 1)
            emit_A(1, 3)
            emit_B(1, 2)
            emit_B(1, 3)
            emit_P(1)

    nc.compile()
    return nc


def _get_nc():
    global _compiled_nc
    if _compiled_nc is None:
        _compiled_nc = _build_nc()
    return _compiled_nc


def _host_tables():
    pos = np.arange(T, dtype=np.float32)[:, None]
    inv = np.exp(np.arange(0, Dh, 2, dtype=np.float32)
                 * (-math.log(10000.0) / Dh))
    ang = pos * inv                       # (T, 32)
    sin, cos = np.sin(ang), np.cos(ang)   # (T, 32)
    idx = np.arange(128) % HALF           # d % 32
    cos_ext = cos[:, idx].T.astype(NPBF16)  # (128, T)
    # Rotation sign baked in, indexed by the INPUT row block of the
    # partition-shifted mul: out rows (0:32, 64:96) need -sin and read
    # input rows (32:64, 96:128); out rows (32:64, 96:128) need +sin and
    # read input rows (0:32, 64:96).
    sign = np.where((np.arange(128) % 64) < HALF, 1.0, -1.0)[:, None]
    sin_ext = (sign * sin[:, idx].T).astype(NPBF16)

    s_i = np.arange(128)[:, None]
    t_i = np.arange(128)[None, :]
    mask01 = (t_i >= s_i).astype(np.float32).astype(NPBF16)
    return cos_ext, sin_ext, mask01


def kernel(x, w_qkv, w_proj):
    x = np.asarray(x)
    w_qkv = np.asarray(w_qkv)
    w_proj = np.asarray(w_proj)
    nc = _get_nc()
    in_maps = build_in_maps(x, w_qkv, w_proj)
    res = run_bass_kernel_spmd(nc, in_maps, core_ids=list(range(NCORES)))
    y = np.zeros((B, T, C), dtype=np.float32)
    for c in range(NCORES):
        b, g = c // 4, c % 4
        y[b, 512 * g:512 * g + 512, :] = res.results[c]["yblk"].astype(
            np.float32)
    return y


def build_in_maps(x, w_qkv, w_proj):
    cos_ext, sin_ext, mask01 = _host_tables()
    wq4 = w_qkv.reshape(3, H, Dh, C)
    wpT = np.ascontiguousarray(w_proj.T.astype(NPBF16))
    in_maps = []
    for c in range(NCORES):
        b, g = c // 4, c % 4
        hs = slice(4 * g, 4 * g + 4)
        wq = wq4[0, hs].reshape(CPC, C)
        wk = wq4[1, hs].reshape(CPC, C)
        wv = wq4[2, hs].reshape(CPC, C)
        wqkvT = np.concatenate([wq, wk, wv], axis=0).T.astype(NPBF16)
        xT = x[b].T.astype(NPBF16)
        in_maps.append({
            "xT": np.ascontiguousarray(xT),
            "wqkvT": np.ascontiguousarray(wqkvT),
            "wpT": wpT,
            "cosx": cos_ext, "sinx": sin_ext,
            "mask01": mask01,
        })
    return in_maps
